# revision 14
# baseline (speedup 1.0000x reference)
"""MoE gating (nn_MoEGate) Trainium2 Bass kernel.

kernel(x, W_g) -> (vals, idx), matching the jax reference:
    logits = x @ W_g.T ; scores = softmax(logits) ; vals, idx = top_k(scores, 8)

Strategy: data-parallel over the token dim — 8 NeuronCores each process a
4096-token shard with W_g replicated; no cross-core communication.

Per-core pipeline (T=4096 tokens, D=4096, E=64 experts, K=8), streamed in
512-token groups:
  - x loaded in natural [token, d] layout (contiguous 16KB/partition DMA
    descriptors, the HBM-optimal pattern), issued on the SP queue.
  - PE transposes 128x128 blocks of x into PSUM in exact fp32.
  - The PSUM->SBUF copy doubles as a fp16 hi/lo split: ACT writes
    xt_hi = fp16(xT), DVE writes xt_lo = fp16(xT - xt_hi). W_g^T chunks are
    split the same way once at setup.
  - Gating matmul in fp16, token-major, 3 products accumulated in PSUM:
    logits ~= xhi*whi + xhi*wlo + xlo*whi. fp16's 11-bit mantissa puts the
    dropped xlo*wlo term at ~2^-24 — fp32-level logits — while fp16 runs
    1 PE cycle/row vs 4 for fp32, so gating is ~4x cheaper than exact fp32
    with (near-)lax.top_k-exact rankings. (bf16 split: ~2e-3/token idx
    flips; single-pass f32r: 314 flips = 2.25e-2 rel-err, over the gate.)
  - Output lands token-major [128t, 64e]: no logits transpose needed.
  - ACT: exp of all 64 logits for the softmax denominator.
  - DVE: native top-8 via InstMax/InstMaxIndex (returns the 8 largest per
    partition in descending order + their indices, lowest-index-first on
    ties — exactly lax.top_k semantics), then vals = exp(top logits)/den.
  - Output DMAs issued on the Pool queue so they never stall x loads.
"""
from contextlib import ExitStack

import numpy as np

import concourse.bacc as bacc
import concourse.mybir as mybir
import concourse.tile as tile
from concourse._compat import with_exitstack
from concourse.bass_utils import run_bass_kernel_spmd

F32 = mybir.dt.float32
F16 = mybir.dt.float16
I32 = mybir.dt.int32
U32 = mybir.dt.uint32
AX = mybir.AxisListType
ALU = mybir.AluOpType
EXP = mybir.ActivationFunctionType.Exp

N_CORES = 8
N_TOKENS = 32768
D = 4096
E = 64
K = 8
T = N_TOKENS // N_CORES  # 4096 tokens per core
NC = D // 128            # 32 d-chunks
GROUP_TOKENS = 512       # 4 token-tiles per group


@with_exitstack
def _moe_gate_kernel(ctx: ExitStack, tc: tile.TileContext, outs, ins,
                     n_tokens=T, repeats=1):
    nc = tc.nc
    x_d, w_d = ins
    vals_d, idx_d = outs
    NG = n_tokens // GROUP_TOKENS
    NPAIR = NC // 2

    consts = ctx.enter_context(tc.tile_pool(name="consts", bufs=1))
    psum = ctx.enter_context(tc.tile_pool(name="psum", bufs=1, space="PSUM"))

    ident128 = consts.tile([128, 128], F32, tag="ident128")
    nc.vector.memset(ident128[:], 1.0)
    nc.gpsimd.affine_select(
        ident128[:], ident128[:], pattern=[[-1, 128]], compare_op=ALU.is_equal,
        fill=0.0, base=0, channel_multiplier=1,
    )
    ident64 = consts.tile([64, 64], F32, tag="ident64")
    nc.vector.memset(ident64[:], 1.0)
    nc.gpsimd.affine_select(
        ident64[:], ident64[:], pattern=[[-1, 64]], compare_op=ALU.is_equal,
        fill=0.0, base=0, channel_multiplier=1,
    )

    # W_g^T chunks in fp16 hi/lo: wt_hi/wt_lo [128 d, c, 64 e]
    wt_hi = consts.tile([128, NC, E], F16, tag="wt_hi")
    wt_lo = consts.tile([128, NC, E], F16, tag="wt_lo")
    with tc.tile_pool(name="wsetup", bufs=1) as wsp:
        w_sb = wsp.tile([64, D], F32, tag="w_sb")
        for q in range(4):
            nc.sync.dma_start(
                w_sb[:, q * (D // 4):(q + 1) * (D // 4)],
                w_d[:, q * (D // 4):(q + 1) * (D // 4)],
            )
        for cq in range(NC // 8):
            wt_ps = psum.tile([128, 1024], F32, tag="xt", bufs=2, name="wt_ps")
            for c8 in range(8):
                c = cq * 8 + c8
                nc.tensor.transpose(
                    wt_ps[:, c8 * 64:(c8 + 1) * 64],
                    w_sb[:, c * 128:(c + 1) * 128],
                    ident64[:],
                )
            sl = wt_hi[:, cq * 8:(cq + 1) * 8, :]
            nc.scalar.copy(sl, wt_ps[:, 0:512])
            nc.vector.tensor_tensor(
                wt_lo[:, cq * 8:(cq + 1) * 8, :], wt_ps[:, 0:512], sl,
                op=ALU.subtract,
            )

    sb = ctx.enter_context(tc.tile_pool(name="main", bufs=1))

    vd = vals_d.rearrange("(t p) k -> p t k", p=128)
    xd = idx_d.rearrange("(t p) k -> p t k", p=128)

    for rep in range(repeats):
        for g in range(NG):
            x_tiles = []
            # Early groups: split loads along d so the first transposes (which
            # only touch low d-chunks) can start after ~2MB instead of 8MB.
            n_splits = 4 if g == 0 else (2 if g == 1 else 1)
            for j in range(4):
                xt = sb.tile([128, D], F32, tag="x", bufs=8, name=f"x_{g}_{j}")
                x_tiles.append(xt)
            dq = D // n_splits
            for q in range(n_splits):
                for j in range(4):
                    r0 = g * GROUP_TOKENS + j * 128
                    nc.sync.dma_start(
                        x_tiles[j][:, q * dq:(q + 1) * dq],
                        x_d[r0:r0 + 128, q * dq:(q + 1) * dq],
                    )

            # token-major logits accumulators: one PSUM bank (2KB zero
            # region) per token-tile accumulation group, logits in the
            # first 64 cols of each bank.
            acc = psum.tile([128, 4, 512], F32, tag="acc", bufs=1, name="acc")

            def gating(pair, xt_hl_t):
                for h in range(2):
                    c = 2 * pair + h
                    for j in range(4):
                        xhi = xt_hl_t[:, 0, h * 512 + j * 128:h * 512 + (j + 1) * 128]
                        xlo = xt_hl_t[:, 1, h * 512 + j * 128:h * 512 + (j + 1) * 128]
                        first = c == 0
                        last = c == NC - 1
                        nc.tensor.matmul(acc[:, j, 0:E], xhi, wt_hi[:, c, :],
                                         start=first, stop=False)
                        nc.tensor.matmul(acc[:, j, 0:E], xhi, wt_lo[:, c, :],
                                         start=False, stop=False)
                        nc.tensor.matmul(acc[:, j, 0:E], xlo, wt_hi[:, c, :],
                                         start=False, stop=last)

            # software pipeline: gating for pair p-SKEW is emitted between the
            # transposes of pair p so the in-order PE stream never waits on the
            # PSUM->SBUF hi/lo split of its own pair.
            SKEW = 2
            xt_hls = {}
            for pair in range(NPAIR):
                xt_ps = psum.tile([128, 1024], F32, tag="xt", bufs=2, name="xt_ps")
                for h in range(2):
                    c = 2 * pair + h
                    for j in range(4):
                        nc.tensor.transpose(
                            xt_ps[:, h * 512 + j * 128:h * 512 + (j + 1) * 128],
                            x_tiles[j][:, c * 128:(c + 1) * 128],
                            ident128[:],
                        )
                xt_hl = sb.tile([128, 2, 1024], F16, tag="xtsb", bufs=4,
                                name="xt_hl")
                nc.scalar.copy(xt_hl[:, 0, :], xt_ps[:])
                nc.vector.tensor_tensor(
                    xt_hl[:, 1, :], xt_ps[:], xt_hl[:, 0, :], op=ALU.subtract
                )
                xt_hls[pair] = xt_hl
                if pair >= SKEW:
                    gating(pair - SKEW, xt_hls.pop(pair - SKEW))
            for pair in range(NPAIR - SKEW, NPAIR):
                gating(pair, xt_hls.pop(pair))

            # ---- group tail: softmax denominator + native top-8 ----
            exp_t = sb.tile([128, 4, E], F32, tag="expt", bufs=2, name="exp_t")
            nc.scalar.activation(exp_t[:], acc[:, :, 0:E], EXP)
            lgt_sb = sb.tile([128, 4, E], F32, tag="lgts", bufs=2, name="lgt_sb")
            nc.vector.tensor_copy(lgt_sb[:], acc[:, :, 0:E])
            den = sb.tile([128, 4], F32, tag="den", bufs=2, name="den")
            nc.vector.reduce_sum(den[:], exp_t[:], axis=AX.X)
            rden = sb.tile([128, 4], F32, tag="rden", bufs=2, name="rden")
            nc.vector.reciprocal(rden[:], den[:])
            mx = sb.tile([128, 4, K], F32, tag="mx", bufs=2, name="mx")
            ixu = sb.tile([128, 4, K], U32, tag="ixu", bufs=2, name="ixu")
            for j in range(4):
                nc.vector.max(mx[:, j, :], lgt_sb[:, j, :])
                nc.vector.max_index(ixu[:, j, :], mx[:, j, :], lgt_sb[:, j, :])
            exk = sb.tile([128, 4, K], F32, tag="exk", bufs=2, name="exk")
            nc.scalar.activation(exk[:], mx[:], EXP)
            vals = sb.tile([128, 4, K], F32, tag="vals", bufs=2, name="vals")
            rden_b = rden[:].unsqueeze(2).broadcast_to([128, 4, K])
            nc.vector.tensor_tensor(vals[:], exk[:], rden_b, op=ALU.mult)

            # out-DMAs on the Pool queue: SP stays dedicated to x loads.
            nc.gpsimd.dma_start(vd[:, g * 4:(g + 1) * 4, :], vals[:])
            nc.gpsimd.dma_start(xd[:, g * 4:(g + 1) * 4, :], ixu[:].bitcast(I32))


_MODEL_CACHE = {}


def build_model(n_tokens=T, repeats=1):
    key = (n_tokens, repeats)
    if key in _MODEL_CACHE:
        return _MODEL_CACHE[key]
    nc = bacc.Bacc(
        "TRN2",
        target_bir_lowering=False,
        debug=False,
        enable_asserts=False,
        num_devices=N_CORES,
    )
    x_d = nc.dram_tensor("x", [n_tokens, D], F32, kind="ExternalInput").ap()
    w_d = nc.dram_tensor("w", [E, D], F32, kind="ExternalInput").ap()
    vals_d = nc.dram_tensor("vals", [n_tokens, K], F32, kind="ExternalOutput").ap()
    idx_d = nc.dram_tensor("idx", [n_tokens, K], I32, kind="ExternalOutput").ap()
    with tile.TileContext(nc) as tc:
        _moe_gate_kernel(tc, [vals_d, idx_d], [x_d, w_d], n_tokens=n_tokens,
                         repeats=repeats)
    nc.compile()
    _MODEL_CACHE[key] = nc
    return nc


def run_on_cores(x, W_g, trace=False, trace_kwargs=None):
    """x [32768, 4096] f32, W_g [64, 4096] f32 -> (vals, idx), plus results obj."""
    nc = build_model()
    x = np.ascontiguousarray(np.asarray(x, dtype=np.float32))
    W_g = np.ascontiguousarray(np.asarray(W_g, dtype=np.float32))
    shards = np.split(x, N_CORES, axis=0)
    in_maps = [{"x": shards[i], "w": W_g} for i in range(N_CORES)]
    res = run_bass_kernel_spmd(
        nc, in_maps, core_ids=list(range(N_CORES)), trace=trace,
        **(trace_kwargs or {}),
    )
    vals = np.concatenate([r["vals"] for r in res.results], axis=0)
    idx = np.concatenate([r["idx"] for r in res.results], axis=0)
    return (vals, idx), res


def kernel(x, W_g):
    (vals, idx), _ = run_on_cores(x, W_g)
    return vals, idx
